# revision 33
# baseline (speedup 1.0000x reference)
"""Causal multi-head self-attention (B=2, S=2048, D=1024, H=16, Dh=64, RoPE)
as a Bass/Tile kernel on 8 Trainium2 NeuronCores.

Sharding: 2-way batch x 4-way head-group tensor parallel.
  core c: batch b = c // 4, head group g = c % 4 (heads 4g..4g+3).
  Wq/Wk/Wv split column-wise by head (rows of W since y = x @ W.T),
  Wo split row-wise; the 4 partial outputs per batch are summed on host.

Per-core layout choices:
  - x is passed transposed (xT: d_model on partitions) so Q^T/K^T come out of
    the projection matmuls directly in (head_dim, seq) layout, which is what
    the scores matmul (contraction over head_dim on partitions) needs.
  - the whole data plane runs in fp16: matmul rate matches fp32r but without
    the N>=256 constraint, DMA traffic halves, and DVE tensor ops hit the
    2x/4x 2-byte fast modes. PSUM accumulation stays fp32.
  - Wq/Wk rows are de-interleaved per head (even pair-elements then odd) so
    RoPE becomes ops on contiguous 32-partition halves. Scores are invariant
    to a consistent permutation of Q/K features, so nothing is un-permuted.
  - scores are computed transposed (keys on partitions, queries free) so the
    P @ V matmul consumes exp(scores) directly with no transposes. Softmax
    skips the max-subtraction (scores are bounded ~|10| here) and the
    denominator comes from a ones-column appended to V.
  - causal mask: fully-masked key blocks are skipped; diagonal blocks are
    masked AFTER exp by a 0/1 lower-triangle fp16 multiply on DVE (cheaper
    than the extra identity@step-matrix matmul on the critical PE).
"""
import sys

sys.path.insert(0, "/opt/trn_rl_repo")

import numpy as np

import concourse.bass as bass
import concourse.tile as tile
from concourse import bacc, mybir
from concourse.bass_utils import run_bass_kernel_spmd

F32 = mybir.dt.float32
F16 = mybir.dt.float16

B = 2
S = 2048
D = 1024
H = 16
DH = 64
NCORES = 8
NGROUPS = 4           # head groups (tensor parallel)
HL = H // NGROUPS     # heads per core = 4
F = HL * DH           # local features per core = 256
SC = 512              # seq chunk (free dim of most matmuls)
NSC = S // SC         # 4
KB = 128              # key block (partition dim of scoresT)
THETA = 10000.0
SWAP16 = list(range(16, 32)) + list(range(16))


def build_nc(repeat=1):
    nc = bacc.Bacc("TRN2", target_bir_lowering=False)

    xT = nc.dram_tensor("xT", (D, S), F16, kind="ExternalInput")
    wqT = nc.dram_tensor("wqT", (D, F), F16, kind="ExternalInput")
    wkT = nc.dram_tensor("wkT", (D, F), F16, kind="ExternalInput")
    wvT = nc.dram_tensor("wvT", (D, F), F16, kind="ExternalInput")
    woT = nc.dram_tensor("woT", (F, D), F16, kind="ExternalInput")
    cc = nc.dram_tensor("cc", (128, S), F16, kind="ExternalInput")
    ss = nc.dram_tensor("ss", (128, S), F16, kind="ExternalInput")
    msk = nc.dram_tensor("msk", (128, SC), F16, kind="ExternalInput")
    out = nc.dram_tensor("out", (S, D), F16, kind="ExternalOutput")

    KC = D // 128  # 8 contraction chunks

    with tile.TileContext(nc) as tc:
        with (
            tc.tile_pool(name="consts", bufs=1) as consts,
            tc.tile_pool(name="persist", bufs=1) as persist,
            tc.tile_pool(name="xs", bufs=2) as xs_pool,
            tc.tile_pool(name="rope", bufs=2) as rope_pool,
            tc.tile_pool(name="pp", bufs=3) as p_pool,
            tc.tile_pool(name="stg", bufs=2) as stg_pool,
            tc.tile_pool(name="og", bufs=2) as out_pool,
            tc.tile_pool(name="mm_ps", bufs=2, space="PSUM") as mm_ps,
            tc.tile_pool(name="sc_ps", bufs=2, space="PSUM") as sc_ps,
            tc.tile_pool(name="pv_ps", bufs=2, space="PSUM") as pv_ps,
        ):
            # ---- constants / weights (ordered by first use) ----
            wq_sb = consts.tile([128, KC, F], F16)
            wk_sb = consts.tile([128, KC, F], F16)
            wv_sb = consts.tile([128, KC, F], F16)
            wo_sb = consts.tile([128, 2, D], F16)
            cc_sb = consts.tile([128, S], F16)
            ss_sb = consts.tile([128, S], F16)
            mk_sb = consts.tile([128, SC], F16)
            xt0 = xs_pool.tile([128, KC, SC], F16, name="xt0", tag="xt")
            # spread the input loads across 4 engine DMA queues so descriptor
            # generation parallelizes; wq/xt0 are k-split so the first Q
            # matmuls start after one k-chunk lands instead of the full load
            for k in range(KC):
                nc.sync.dma_start(out=wq_sb[:, k, :],
                                  in_=wqT[k * 128:(k + 1) * 128, :])
                nc.gpsimd.dma_start(out=xt0[:, k, :],
                                    in_=xT[k * 128:(k + 1) * 128, 0:SC])
            # wk split across Act (k0-3) + SP (k4-7) so K matmuls start right
            # after Q; wv k-split lands on Pool as the V matmuls consume it
            for k in range(4):
                nc.scalar.dma_start(out=wk_sb[:, k, :],
                                    in_=wkT[k * 128:(k + 1) * 128, :])
            for k in range(4, KC):
                nc.sync.dma_start(out=wk_sb[:, k, :],
                                  in_=wkT[k * 128:(k + 1) * 128, :])
            for k in range(KC):
                nc.gpsimd.dma_start(out=wv_sb[:, k, :],
                                    in_=wvT[k * 128:(k + 1) * 128, :])
            nc.scalar.dma_start(out=cc_sb[:, 0:SC], in_=cc[:, 0:SC])
            nc.scalar.dma_start(out=ss_sb[:, 0:SC], in_=ss[:, 0:SC])
            nc.scalar.dma_start(out=cc_sb[:, SC:], in_=cc[:, SC:])
            nc.scalar.dma_start(out=ss_sb[:, SC:], in_=ss[:, SC:])
            nc.gpsimd.dma_start(out=mk_sb, in_=msk[:, :])
            nc.sync.dma_start(out=wo_sb, in_=woT.rearrange("(ft p) d -> p ft d", p=128))

            # ---- persistent activations ----
            qtr = [persist.tile([128, S], F16, name=f"qtr{i}", tag=f"qtr{i}") for i in range(2)]
            ktr = [persist.tile([128, S], F16, name=f"ktr{i}", tag=f"ktr{i}") for i in range(2)]
            # aot is per-(chunk, ft) so wo(sc) never picks up a false
            # dependency on attn(sc+1)'s writes to other chunks
            aot = [[persist.tile([128, SC], F16, name=f"aot{i}_{sc_}",
                                 tag=f"aot{i}_{sc_}") for i in range(2)]
                   for sc_ in range(NSC)]
            # V_ext: 16 seq tiles of (128, 4 heads * 65); col 65h+64 is the
            # ones column that produces the softmax denominator in P @ V_ext.
            vext = [persist.tile([128, HL * 65], F16, name=f"vext{i}", tag=f"vext{i}")
                    for i in range(S // 128)]
            for v in vext:
                nc.gpsimd.memset(
                    v.rearrange("p (h c) -> p h c", c=65)[:, :, 64:65], 1.0)

            def emit_qk(sc, xt, ft):
                """One feature-tile of the Q and K projections + RoPE."""
                s0 = sc * SC
                for w_sb, dst in ((wq_sb, qtr), (wk_sb, ktr)):
                    ps = mm_ps.tile([128, SC], F32, name="ps", tag="mm")
                    for k in range(KC):
                        nc.tensor.matmul(ps, w_sb[:, k, ft * 128:(ft + 1) * 128],
                                         xt[:, k, :],
                                         start=(k == 0), stop=(k == KC - 1))
                    qf = rope_pool.tile([128, SC], F16, name="qf", tag="qf")
                    nc.scalar.copy(out=qf, in_=ps)
                    qsw = rope_pool.tile([128, SC], F16, name="qsw", tag="qsw")
                    nc.vector.stream_shuffle(qsw, qf, SWAP16)
                    dslice = dst[ft][:, s0:s0 + SC]
                    nc.vector.tensor_mul(qsw, qsw, ss_sb[:, s0:s0 + SC])
                    nc.vector.tensor_mul(dslice, qf, cc_sb[:, s0:s0 + SC])
                    nc.vector.tensor_add(dslice, dslice, qsw)

            def emit_v(sc, xt, sts):
                for st in sts:
                    psv = mm_ps.tile([128, F], F32, name="psv", tag="mm")
                    for k in range(KC):
                        nc.tensor.matmul(psv, xt[:, k, st * 128:(st + 1) * 128],
                                         wv_sb[:, k, :],
                                         start=(k == 0), stop=(k == KC - 1))
                    v = vext[sc * 4 + st]
                    v3 = v.rearrange("p (h c) -> p h c", c=65)[:, :, 0:DH]
                    p3 = psv.rearrange("p (h c) -> p h c", c=DH)
                    nc.vector.tensor_copy(out=v3, in_=p3)

            def emit_proj(sc, xt, first=False):
                """Q/K projections + RoPE and V projection for chunk sc.

                For the first chunk (nothing to overlap with), interleave so
                RoPE of ft0 runs while the PE does ft1/V work."""
                if first:
                    emit_qk(sc, xt, 0)
                    emit_v(sc, xt, (0, 1))
                    emit_qk(sc, xt, 1)
                    emit_v(sc, xt, (2, 3))
                else:
                    emit_qk(sc, xt, 0)
                    emit_qk(sc, xt, 1)
                    emit_v(sc, xt, (0, 1, 2, 3))

            def emit_prefetch(sc):
                xtn = xs_pool.tile([128, KC, SC], F16, name="xtn", tag="xt")
                nc.sync.dma_start(
                    out=xtn,
                    in_=xT[:, sc * SC:(sc + 1) * SC].rearrange(
                        "(kc p) s -> p kc s", p=128))
                return xtn

            def emit_attn(sc, horder=(1, 3, 0, 2), pre_norm_cb=None):
                s0 = sc * SC
                nblocks = 4 * sc + 4
                for hi, h in enumerate(horder):  # even heads last: their norm
                    ft, hr = h // 2, (h % 2) * DH      # writes skip stg DMA
                    pv = pv_ps.tile([65, SC], F32, name="pv", tag="pv")
                    for pair in range(nblocks // 2):
                        sps = sc_ps.tile([128, 2 * SC], F32, name="sps", tag="sps")
                        w = []
                        for half in range(2):
                            j = 2 * pair + half
                            diag = j >= 4 * sc
                            t = j - 4 * sc
                            w0 = 128 * t if diag else 0
                            w.append(w0)
                            tgt = sps[:, half * SC + w0:(half + 1) * SC]
                            nc.tensor.matmul(tgt,
                                             ktr[ft][hr:hr + DH, j * 128:(j + 1) * 128],
                                             qtr[ft][hr:hr + DH, s0 + w0:s0 + SC],
                                             start=True, stop=True)
                        pt = p_pool.tile([128, 2 * SC], F16, name="pt", tag="pt")
                        if w[0] == 0 and w[1] == 0:
                            nc.scalar.activation(out=pt, in_=sps,
                                                 func=mybir.ActivationFunctionType.Exp,
                                                 scale=0.125)
                        else:
                            for half in range(2):
                                sl = slice(half * SC + w[half], (half + 1) * SC)
                                nc.scalar.activation(out=pt[:, sl], in_=sps[:, sl],
                                                     func=mybir.ActivationFunctionType.Exp,
                                                     scale=0.125)
                        for half in range(2):
                            j = 2 * pair + half
                            if j >= 4 * sc:  # diagonal: zero the upper triangle
                                w0 = w[half]
                                sl = slice(half * SC + w0, (half + 1) * SC)
                                nc.vector.tensor_mul(pt[:, sl], pt[:, sl],
                                                     mk_sb[:, 0:SC - w0])
                        for half in range(2):
                            j = 2 * pair + half
                            w0 = w[half]
                            nc.tensor.matmul(pv[:, w0:SC],
                                             vext[j][:, h * 65:h * 65 + 65],
                                             pt[:, half * SC + w0:(half + 1) * SC],
                                             start=(j == 0), stop=(j == nblocks - 1))
                    # normalize: aot_norm = pv[0:64] * (1 / D) broadcast over
                    # rows. HW partition_broadcast replicates the TILE's
                    # partition 0 (it ignores the AP offset), so first shift
                    # the reciprocal row to a partition-0 tile on GPSIMD.
                    if hi == len(horder) - 1 and pre_norm_cb is not None:
                        pre_norm_cb()
                    rb = stg_pool.tile([65, SC], F32, name="rb", tag="rb")
                    nc.vector.reciprocal(out=rb[64:65, :], in_=pv[64:65, :])
                    r1 = stg_pool.tile([1, SC], F32, name="r1", tag="r1")
                    nc.gpsimd.tensor_copy(out=r1, in_=rb[64:65, :])
                    bcs = stg_pool.tile([DH, SC], F32, name="bcs", tag="bcs")
                    nc.gpsimd.partition_broadcast(bcs, r1[0:1, :])
                    if hr == 0:
                        nc.vector.tensor_mul(aot[sc][ft][0:DH, :], pv[0:DH, :], bcs)
                    else:
                        stg = stg_pool.tile([DH, SC], F16, name="stg", tag="stg")
                        nc.vector.tensor_mul(stg, pv[0:DH, :], bcs)
                        nc.sync.dma_start(out=aot[sc][ft][hr:hr + DH, :], in_=stg)

            def emit_wo(sc, spread_dma=False):
                s0 = sc * SC
                for st in range(4):
                    so = s0 + st * 128
                    og = out_pool.tile([128, D], F16, name="og", tag="og")
                    pws = []
                    for nn in range(2):
                        pw = mm_ps.tile([128, SC], F32, name="pw", tag="mm")
                        for ft in range(2):
                            nc.tensor.matmul(pw, aot[sc][ft][:, st * 128:(st + 1) * 128],
                                             wo_sb[:, ft, nn * SC:(nn + 1) * SC],
                                             start=(ft == 0), stop=(ft == 1))
                        pws.append(pw)
                    # copies on two engines in parallel so the mm_ps bufs
                    # recycle fast enough to keep the PE fed (GPSIMD cannot
                    # read PSUM on real HW)
                    nc.vector.tensor_copy(out=og[:, 0:SC], in_=pws[0])
                    nc.vector.tensor_copy(out=og[:, SC:2 * SC], in_=pws[1])
                    q = (nc.sync, nc.sync, nc.scalar, nc.gpsimd)[st] if spread_dma \
                        else nc.sync
                    q.dma_start(out=out[so:so + 128, :], in_=og)

            # schedule: next chunk's projections run between attention(sc)
            # and Wo(sc-1) so the PE has work while the normalization chains
            # drain; wo(2) lands right after attn(3) to cover its last norm.
            xt_first = xt0
            for rep in range(repeat):
                emit_proj(0, xt_first, first=(rep == 0))
                xt_next = emit_prefetch(1)
                for sc in range(NSC):
                    if sc == NSC - 1:
                        # wo(sc-1) emits between the last head's PV and its
                        # normalization so the PE chews on it while the norm
                        # chain drains
                        emit_attn(sc, pre_norm_cb=lambda s=sc: emit_wo(s - 1))
                    else:
                        emit_attn(sc)
                        xt_cur = xt_next
                        if sc + 2 < NSC:
                            xt_next = emit_prefetch(sc + 2)
                        emit_proj(sc + 1, xt_cur)
                        if sc >= 1:
                            emit_wo(sc - 1)
                emit_wo(NSC - 1, spread_dma=True)
                if rep + 1 < repeat:
                    xt_first = emit_prefetch(0)

    nc.compile()
    return nc


def _rope_tables():
    inv_freq = 1.0 / (THETA ** (np.arange(0, DH, 2, dtype=np.float64) / DH))  # (32,)
    ang = np.arange(S, dtype=np.float64)[:, None] * inv_freq[None, :]         # (S, 32)
    cos = np.cos(ang).T.astype(np.float32)                                    # (32, S)
    sin = np.sin(ang).T.astype(np.float32)
    # quadrant layout per head: [x1(f0:16); x2(f0:16); x1(f16:32); x2(f16:32)]
    cc64 = np.concatenate([cos[0:16], cos[0:16], cos[16:32], cos[16:32]], axis=0)
    ss64 = np.concatenate([-sin[0:16], sin[0:16], -sin[16:32], sin[16:32]], axis=0)
    cc = np.tile(cc64, (2, 1)).astype(np.float16)                             # (128, S)
    ss = np.tile(ss64, (2, 1)).astype(np.float16)
    return np.ascontiguousarray(cc), np.ascontiguousarray(ss)


def _mask():
    # msk[k, c] = 1 if c >= k else 0  (lower triangle for diagonal blocks)
    k = np.arange(128)[:, None]
    c = np.arange(SC)[None, :]
    return (c >= k).astype(np.float16)


def _perm_rows():
    # per head, per 32-row quadrant: 16 even pair-elements then their odds,
    # so the RoPE partner swap stays within a 32-partition stream_shuffle group
    p = []
    for h in range(HL):
        base = h * DH
        p.extend(base + np.arange(0, 32, 2))   # x1 of pairs 0..15
        p.extend(base + np.arange(1, 32, 2))   # x2 of pairs 0..15
        p.extend(base + 32 + np.arange(0, 32, 2))  # x1 of pairs 16..31
        p.extend(base + 32 + np.arange(1, 32, 2))  # x2 of pairs 16..31
    return np.array(p)


_NC_CACHE = {}


def make_in_maps(x, Wq, Wk, Wv, Wo):
    x = np.asarray(x, dtype=np.float32)
    Wq = np.asarray(Wq, dtype=np.float32)
    Wk = np.asarray(Wk, dtype=np.float32)
    Wv = np.asarray(Wv, dtype=np.float32)
    Wo = np.asarray(Wo, dtype=np.float32)

    cc, ss = _rope_tables()
    mk = _mask()
    perm = _perm_rows()

    in_maps = []
    for c in range(NCORES):
        b, g = c // NGROUPS, c % NGROUPS
        rows = slice(g * F, (g + 1) * F)
        wq_g = Wq[rows, :][perm, :]
        wk_g = Wk[rows, :][perm, :]
        in_maps.append({
            "xT": np.ascontiguousarray(x[b].T.astype(np.float16)),
            "wqT": np.ascontiguousarray(wq_g.T.astype(np.float16)),
            "wkT": np.ascontiguousarray(wk_g.T.astype(np.float16)),
            "wvT": np.ascontiguousarray(Wv[rows, :].T.astype(np.float16)),
            "woT": np.ascontiguousarray(Wo[:, rows].T.astype(np.float16)),
            "cc": cc, "ss": ss, "msk": mk,
        })
    return in_maps


def kernel(x, Wq, Wk, Wv, Wo):
    in_maps = make_in_maps(x, Wq, Wk, Wv, Wo)

    if "nc" not in _NC_CACHE:
        _NC_CACHE["nc"] = build_nc()
    nc = _NC_CACHE["nc"]
    res = run_bass_kernel_spmd(nc, in_maps, core_ids=list(range(NCORES)))

    out = np.zeros((B, S, D), dtype=np.float64)
    for c in range(NCORES):
        out[c // NGROUPS] += res.results[c]["out"].astype(np.float64)
    return out.astype(np.float32)


# revision 39
# speedup vs baseline: 1.0147x; 1.0147x over previous
"""Causal multi-head self-attention (B=2, S=2048, D=1024, H=16, Dh=64, RoPE)
as a Bass/Tile kernel on 8 Trainium2 NeuronCores.

Sharding: 2-way batch x 4-way head-group tensor parallel.
  core c: batch b = c // 4, head group g = c % 4 (heads 4g..4g+3).
  Wq/Wk/Wv split column-wise by head (rows of W since y = x @ W.T),
  Wo split row-wise; the 4 partial outputs per batch are summed on host.

Per-core layout choices:
  - x is passed transposed (xT: d_model on partitions) so Q^T/K^T come out of
    the projection matmuls directly in (head_dim, seq) layout, which is what
    the scores matmul (contraction over head_dim on partitions) needs.
  - the whole data plane runs in fp16: matmul rate matches fp32r but without
    the N>=256 constraint, DMA traffic halves, and DVE tensor ops hit the
    2x/4x 2-byte fast modes. PSUM accumulation stays fp32.
  - Wq/Wk rows are de-interleaved per head (even pair-elements then odd) so
    RoPE becomes ops on contiguous 32-partition halves. Scores are invariant
    to a consistent permutation of Q/K features, so nothing is un-permuted.
  - scores are computed transposed (keys on partitions, queries free) so the
    P @ V matmul consumes exp(scores) directly with no transposes. Softmax
    skips the max-subtraction (scores are bounded ~|10| here) and the
    denominator comes from a ones-column appended to V.
  - causal mask: fully-masked key blocks are skipped; diagonal blocks get
    -60000 (an fp16-safe -inf: exp underflows to exactly 0) written at full
    half-width via an identity @ step-matrix matmul that STARTS the PSUM
    accumulation, with the scores matmul accumulating on top. Writing the
    full width keeps every sps element defined so one fused exp per block
    pair suffices (half the Act instruction count).
  - softmax normalization: reciprocal of the ones-column row, broadcast via
    a DRAM bounce (partition-stride-0 DRAM reads are the only legal
    partition broadcast; GPSIMD partition_broadcast ignores AP partition
    offsets on real HW), then one multiply per head.
"""
import sys

sys.path.insert(0, "/opt/trn_rl_repo")

import numpy as np

import concourse.bass as bass
import concourse.tile as tile
from concourse import bacc, mybir
from concourse.bass_utils import run_bass_kernel_spmd

F32 = mybir.dt.float32
F16 = mybir.dt.float16

B = 2
S = 2048
D = 1024
H = 16
DH = 64
NCORES = 8
NGROUPS = 4           # head groups (tensor parallel)
HL = H // NGROUPS     # heads per core = 4
F = HL * DH           # local features per core = 256
SC = 512              # seq chunk (free dim of most matmuls)
NSC = S // SC         # 4
KB = 128              # key block (partition dim of scoresT)
THETA = 10000.0
SWAP16 = list(range(16, 32)) + list(range(16))


def build_nc(repeat=1):
    nc = bacc.Bacc("TRN2", target_bir_lowering=False)

    xT = nc.dram_tensor("xT", (D, S), F16, kind="ExternalInput")
    wqT = nc.dram_tensor("wqT", (D, F), F16, kind="ExternalInput")
    wkT = nc.dram_tensor("wkT", (D, F), F16, kind="ExternalInput")
    wvT = nc.dram_tensor("wvT", (D, F), F16, kind="ExternalInput")
    woT = nc.dram_tensor("woT", (F, D), F16, kind="ExternalInput")
    cc = nc.dram_tensor("cc", (128, S), F16, kind="ExternalInput")
    ss = nc.dram_tensor("ss", (128, S), F16, kind="ExternalInput")
    gmask = nc.dram_tensor("gmask", (128, 2 * SC), F16, kind="ExternalInput")
    eye = nc.dram_tensor("eye", (128, 128), F16, kind="ExternalInput")
    out = nc.dram_tensor("out", (S, D), F16, kind="ExternalOutput")

    KC = D // 128  # 8 contraction chunks

    with tile.TileContext(nc) as tc:
        with (
            tc.tile_pool(name="consts", bufs=1) as consts,
            tc.tile_pool(name="persist", bufs=1) as persist,
            tc.tile_pool(name="xs", bufs=2) as xs_pool,
            tc.tile_pool(name="rope", bufs=2) as rope_pool,
            tc.tile_pool(name="pp", bufs=3) as p_pool,
            tc.tile_pool(name="stg", bufs=2) as stg_pool,
            tc.tile_pool(name="og", bufs=2) as out_pool,
            tc.tile_pool(name="dscr", bufs=2, space="DRAM") as dscr_pool,
            tc.tile_pool(name="mm_ps", bufs=2, space="PSUM") as mm_ps,
            tc.tile_pool(name="sc_ps", bufs=2, space="PSUM") as sc_ps,
            tc.tile_pool(name="pv_ps", bufs=2, space="PSUM") as pv_ps,
        ):
            # ---- constants / weights (ordered by first use) ----
            wq_sb = consts.tile([128, KC, F], F16)
            wk_sb = consts.tile([128, KC, F], F16)
            wv_sb = consts.tile([128, KC, F], F16)
            wo_sb = consts.tile([128, 2, D], F16)
            cc_sb = consts.tile([128, S], F16)
            ss_sb = consts.tile([128, S], F16)
            gm_sb = consts.tile([128, 2 * SC], F16)
            eye_sb = consts.tile([128, 128], F16)
            xt0 = xs_pool.tile([128, KC, SC], F16, name="xt0", tag="xt")
            # spread the input loads across 4 engine DMA queues so descriptor
            # generation parallelizes; wq/xt0 are k-split so the first Q
            # matmuls start after one k-chunk lands instead of the full load
            for k in range(KC):
                nc.sync.dma_start(out=wq_sb[:, k, :],
                                  in_=wqT[k * 128:(k + 1) * 128, :])
                nc.gpsimd.dma_start(out=xt0[:, k, :],
                                    in_=xT[k * 128:(k + 1) * 128, 0:SC])
            # wk split across Act (k0-3) + SP (k4-7) so K matmuls start right
            # after Q; wv k-split lands on Pool as the V matmuls consume it
            for k in range(4):
                nc.scalar.dma_start(out=wk_sb[:, k, :],
                                    in_=wkT[k * 128:(k + 1) * 128, :])
            for k in range(4, KC):
                nc.sync.dma_start(out=wk_sb[:, k, :],
                                  in_=wkT[k * 128:(k + 1) * 128, :])
            for k in range(KC):
                nc.gpsimd.dma_start(out=wv_sb[:, k, :],
                                    in_=wvT[k * 128:(k + 1) * 128, :])
            nc.scalar.dma_start(out=cc_sb[:, 0:SC], in_=cc[:, 0:SC])
            nc.scalar.dma_start(out=ss_sb[:, 0:SC], in_=ss[:, 0:SC])
            nc.scalar.dma_start(out=cc_sb[:, SC:], in_=cc[:, SC:])
            nc.scalar.dma_start(out=ss_sb[:, SC:], in_=ss[:, SC:])
            nc.gpsimd.dma_start(out=gm_sb, in_=gmask[:, :])
            nc.gpsimd.dma_start(out=eye_sb, in_=eye[:, :])
            nc.sync.dma_start(out=wo_sb, in_=woT.rearrange("(ft p) d -> p ft d", p=128))

            # ---- persistent activations ----
            qtr = [persist.tile([128, S], F16, name=f"qtr{i}", tag=f"qtr{i}") for i in range(2)]
            ktr = [persist.tile([128, S], F16, name=f"ktr{i}", tag=f"ktr{i}") for i in range(2)]
            # aot is per-(chunk, ft) so wo(sc) never picks up a false
            # dependency on attn(sc+1)'s writes to other chunks
            aot = [[persist.tile([128, SC], F16, name=f"aot{i}_{sc_}",
                                 tag=f"aot{i}_{sc_}") for i in range(2)]
                   for sc_ in range(NSC)]
            # V_ext: 16 seq tiles of (128, 4 heads * 65); col 65h+64 is the
            # ones column that produces the softmax denominator in P @ V_ext.
            vext = [persist.tile([128, HL * 65], F16, name=f"vext{i}", tag=f"vext{i}")
                    for i in range(S // 128)]
            for v in vext:
                nc.gpsimd.memset(
                    v.rearrange("p (h c) -> p h c", c=65)[:, :, 64:65], 1.0)

            def emit_qk(sc, xt, ft):
                """One feature-tile of the Q and K projections + RoPE."""
                s0 = sc * SC
                for w_sb, dst in ((wq_sb, qtr), (wk_sb, ktr)):
                    ps = mm_ps.tile([128, SC], F32, name="ps", tag="mm")
                    for k in range(KC):
                        nc.tensor.matmul(ps, w_sb[:, k, ft * 128:(ft + 1) * 128],
                                         xt[:, k, :],
                                         start=(k == 0), stop=(k == KC - 1))
                    qf = rope_pool.tile([128, SC], F16, name="qf", tag="qf")
                    nc.vector.tensor_copy(out=qf, in_=ps)
                    qsw = rope_pool.tile([128, SC], F16, name="qsw", tag="qsw")
                    nc.vector.stream_shuffle(qsw, qf, SWAP16)
                    dslice = dst[ft][:, s0:s0 + SC]
                    nc.vector.tensor_mul(qsw, qsw, ss_sb[:, s0:s0 + SC])
                    nc.vector.tensor_mul(dslice, qf, cc_sb[:, s0:s0 + SC])
                    nc.vector.tensor_add(dslice, dslice, qsw)

            def emit_v(sc, xt, sts):
                for st in sts:
                    psv = mm_ps.tile([128, F], F32, name="psv", tag="mm")
                    for k in range(KC):
                        nc.tensor.matmul(psv, xt[:, k, st * 128:(st + 1) * 128],
                                         wv_sb[:, k, :],
                                         start=(k == 0), stop=(k == KC - 1))
                    v = vext[sc * 4 + st]
                    v3 = v.rearrange("p (h c) -> p h c", c=65)[:, :, 0:DH]
                    p3 = psv.rearrange("p (h c) -> p h c", c=DH)
                    nc.vector.tensor_copy(out=v3, in_=p3)

            def emit_proj(sc, xt, first=False):
                """Q/K projections + RoPE and V projection for chunk sc.

                For the first chunk (nothing to overlap with), interleave so
                RoPE of ft0 runs while the PE does ft1/V work."""
                if first:
                    emit_qk(sc, xt, 0)
                    emit_v(sc, xt, (0, 1))
                    emit_qk(sc, xt, 1)
                    emit_v(sc, xt, (2, 3))
                else:
                    emit_qk(sc, xt, 0)
                    emit_qk(sc, xt, 1)
                    emit_v(sc, xt, (0, 1, 2, 3))

            def emit_prefetch(sc):
                xtn = xs_pool.tile([128, KC, SC], F16, name="xtn", tag="xt")
                nc.sync.dma_start(
                    out=xtn,
                    in_=xT[:, sc * SC:(sc + 1) * SC].rearrange(
                        "(kc p) s -> p kc s", p=128))
                return xtn

            def emit_attn(sc, horder=(1, 3, 0, 2), pre_norm_cb=None):
                s0 = sc * SC
                nblocks = 4 * sc + 4
                for hi, h in enumerate(horder):  # even heads last: their norm
                    ft, hr = h // 2, (h % 2) * DH      # writes skip stg DMA
                    pv = pv_ps.tile([65, SC], F32, name="pv", tag="pv")
                    for pair in range(nblocks // 2):
                        sps = sc_ps.tile([128, 2 * SC], F32, name="sps", tag="sps")
                        w = []
                        for half in range(2):
                            j = 2 * pair + half
                            diag = j >= 4 * sc
                            t = j - 4 * sc
                            w0 = 128 * t if diag else 0
                            w.append(w0)
                            tgt = sps[:, half * SC + w0:(half + 1) * SC]
                            if diag:
                                # write -60000 above the diagonal first, at
                                # full half-width so every element of sps is
                                # defined each round (the fused exp reads the
                                # whole tile); scores then accumulate on top
                                nc.tensor.matmul(sps[:, half * SC:(half + 1) * SC],
                                                 eye_sb,
                                                 gm_sb[:, SC - 128 * t:2 * SC - 128 * t],
                                                 start=True, stop=False)
                            nc.tensor.matmul(tgt,
                                             ktr[ft][hr:hr + DH, j * 128:(j + 1) * 128],
                                             qtr[ft][hr:hr + DH, s0 + w0:s0 + SC],
                                             start=not diag, stop=True)
                        pt = p_pool.tile([128, 2 * SC], F16, name="pt", tag="pt")
                        nc.scalar.activation(out=pt, in_=sps,
                                             func=mybir.ActivationFunctionType.Exp,
                                             scale=0.125)
                        for half in range(2):
                            j = 2 * pair + half
                            w0 = w[half]
                            nc.tensor.matmul(pv[:, w0:SC],
                                             vext[j][:, h * 65:h * 65 + 65],
                                             pt[:, half * SC + w0:(half + 1) * SC],
                                             start=(j == 0), stop=(j == nblocks - 1))
                    # normalize: aot_norm = pv[0:64] * (1 / D) broadcast over
                    # rows. HW partition_broadcast replicates the TILE's
                    # partition 0 (it ignores the AP offset), so first shift
                    # the reciprocal row to a partition-0 tile on GPSIMD.
                    if hi == len(horder) - 1 and pre_norm_cb is not None:
                        pre_norm_cb()
                    rb = stg_pool.tile([65, SC], F32, name="rb", tag="rb")
                    nc.vector.reciprocal(out=rb[64:65, :], in_=pv[64:65, :])
                    dscr = dscr_pool.tile([1, SC], F32, name="dscr", tag="dscr")
                    nc.sync.dma_start(out=dscr, in_=rb[64:65, :])
                    bcs = stg_pool.tile([DH, SC], F32, name="bcs", tag="bcs")
                    rb_bcast = bass.AP(tensor=dscr.tensor, offset=dscr.offset,
                                       ap=[[0, DH]] + list(dscr.ap[1:]))
                    nc.sync.dma_start(out=bcs, in_=rb_bcast)
                    if hr == 0:
                        nc.vector.tensor_mul(aot[sc][ft][0:DH, :], pv[0:DH, :], bcs)
                    else:
                        stg = stg_pool.tile([DH, SC], F16, name="stg", tag="stg")
                        nc.vector.tensor_mul(stg, pv[0:DH, :], bcs)
                        nc.sync.dma_start(out=aot[sc][ft][hr:hr + DH, :], in_=stg)

            def emit_wo(sc, spread_dma=False):
                s0 = sc * SC
                for st in range(4):
                    so = s0 + st * 128
                    og = out_pool.tile([128, D], F16, name="og", tag="og")
                    pws = []
                    for nn in range(2):
                        pw = mm_ps.tile([128, SC], F32, name="pw", tag="mm")
                        for ft in range(2):
                            nc.tensor.matmul(pw, aot[sc][ft][:, st * 128:(st + 1) * 128],
                                             wo_sb[:, ft, nn * SC:(nn + 1) * SC],
                                             start=(ft == 0), stop=(ft == 1))
                        pws.append(pw)
                    # copies on two engines in parallel so the mm_ps bufs
                    # recycle fast enough to keep the PE fed (GPSIMD cannot
                    # read PSUM on real HW)
                    nc.vector.tensor_copy(out=og[:, 0:SC], in_=pws[0])
                    nc.scalar.copy(out=og[:, SC:2 * SC], in_=pws[1])
                    q = (nc.sync, nc.sync, nc.scalar, nc.gpsimd)[st] if spread_dma \
                        else nc.sync
                    q.dma_start(out=out[so:so + 128, :], in_=og)

            # schedule: next chunk's projections run between attention(sc)
            # and Wo(sc-1) so the PE has work while the normalization chains
            # drain; wo(2) lands right after attn(3) to cover its last norm.
            xt_first = xt0
            for rep in range(repeat):
                emit_proj(0, xt_first, first=(rep == 0))
                xt_next = emit_prefetch(1)
                for sc in range(NSC):
                    if sc == NSC - 1:
                        # wo(sc-1) emits between the last head's PV and its
                        # normalization so the PE chews on it while the norm
                        # chain drains
                        emit_attn(sc, pre_norm_cb=lambda s=sc: emit_wo(s - 1))
                    else:
                        emit_attn(sc)
                        xt_cur = xt_next
                        if sc + 2 < NSC:
                            xt_next = emit_prefetch(sc + 2)
                        emit_proj(sc + 1, xt_cur)
                        if sc >= 1:
                            emit_wo(sc - 1)
                emit_wo(NSC - 1, spread_dma=True)
                if rep + 1 < repeat:
                    xt_first = emit_prefetch(0)

    nc.compile()
    return nc


def _rope_tables():
    inv_freq = 1.0 / (THETA ** (np.arange(0, DH, 2, dtype=np.float64) / DH))  # (32,)
    ang = np.arange(S, dtype=np.float64)[:, None] * inv_freq[None, :]         # (S, 32)
    cos = np.cos(ang).T.astype(np.float32)                                    # (32, S)
    sin = np.sin(ang).T.astype(np.float32)
    # quadrant layout per head: [x1(f0:16); x2(f0:16); x1(f16:32); x2(f16:32)]
    cc64 = np.concatenate([cos[0:16], cos[0:16], cos[16:32], cos[16:32]], axis=0)
    ss64 = np.concatenate([-sin[0:16], sin[0:16], -sin[16:32], sin[16:32]], axis=0)
    cc = np.tile(cc64, (2, 1)).astype(np.float16)                             # (128, S)
    ss = np.tile(ss64, (2, 1)).astype(np.float16)
    return np.ascontiguousarray(cc), np.ascontiguousarray(ss)


NEG16 = -60000.0  # "-inf" that fits fp16; exp(0.125 * -6e4) underflows to 0


def _gmask():
    # gm[j, c] = NEG16 if j > c - SC else 0   (c in [0, 2*SC))
    j = np.arange(128)[:, None]
    c = np.arange(2 * SC)[None, :]
    return np.where(j > c - SC, np.float16(NEG16), np.float16(0.0))


def _perm_rows():
    # per head, per 32-row quadrant: 16 even pair-elements then their odds,
    # so the RoPE partner swap stays within a 32-partition stream_shuffle group
    p = []
    for h in range(HL):
        base = h * DH
        p.extend(base + np.arange(0, 32, 2))   # x1 of pairs 0..15
        p.extend(base + np.arange(1, 32, 2))   # x2 of pairs 0..15
        p.extend(base + 32 + np.arange(0, 32, 2))  # x1 of pairs 16..31
        p.extend(base + 32 + np.arange(1, 32, 2))  # x2 of pairs 16..31
    return np.array(p)


_NC_CACHE = {}


def make_in_maps(x, Wq, Wk, Wv, Wo):
    x = np.asarray(x, dtype=np.float32)
    Wq = np.asarray(Wq, dtype=np.float32)
    Wk = np.asarray(Wk, dtype=np.float32)
    Wv = np.asarray(Wv, dtype=np.float32)
    Wo = np.asarray(Wo, dtype=np.float32)

    cc, ss = _rope_tables()
    gm = _gmask()
    eye16 = np.eye(128, dtype=np.float16)
    perm = _perm_rows()

    in_maps = []
    for c in range(NCORES):
        b, g = c // NGROUPS, c % NGROUPS
        rows = slice(g * F, (g + 1) * F)
        wq_g = Wq[rows, :][perm, :]
        wk_g = Wk[rows, :][perm, :]
        in_maps.append({
            "xT": np.ascontiguousarray(x[b].T.astype(np.float16)),
            "wqT": np.ascontiguousarray(wq_g.T.astype(np.float16)),
            "wkT": np.ascontiguousarray(wk_g.T.astype(np.float16)),
            "wvT": np.ascontiguousarray(Wv[rows, :].T.astype(np.float16)),
            "woT": np.ascontiguousarray(Wo[:, rows].T.astype(np.float16)),
            "cc": cc, "ss": ss, "gmask": gm, "eye": eye16,
        })
    return in_maps


def kernel(x, Wq, Wk, Wv, Wo):
    in_maps = make_in_maps(x, Wq, Wk, Wv, Wo)

    if "nc" not in _NC_CACHE:
        _NC_CACHE["nc"] = build_nc()
    nc = _NC_CACHE["nc"]
    res = run_bass_kernel_spmd(nc, in_maps, core_ids=list(range(NCORES)))

    out = np.zeros((B, S, D), dtype=np.float64)
    for c in range(NCORES):
        out[c // NGROUPS] += res.results[c]["out"].astype(np.float64)
    return out.astype(np.float32)
